# revision 9
# baseline (speedup 1.0000x reference)
"""AttentionHead kernel for 8 Trainium2 NeuronCores.

Problem: x[4,2048,1024] -> Q/K/V projections (qkv_dim=128) -> softmax(Q K^T / sqrt(128)) @ V.

Sharding: core c handles batch b=c//2, query half h=c%2 (1024 queries), with the
full 2048-key sequence for that batch kept local (data-parallel over batch x
query-split; the SxS score matrix stays on-core per the hint). K/V rows are
processed in the order [this core's query half, other half] - softmax and the
attention-weighted sum are permutation-invariant over keys, so each core can
consume the two halves in its own order and no re-indexing is needed.
"""

import sys

if "/opt/trn_rl_repo" not in sys.path:
    sys.path.insert(0, "/opt/trn_rl_repo")

import numpy as np

P = 128
D = 1024  # d_model
DC = D // P  # 8 contraction chunks
E = 128  # qkv dim
SQ = 1024  # queries per core
SK = 2048  # keys per core
QT = 512  # query column-block width
NQT = SQ // QT  # 2
NKC = SK // P  # 16 key chunks
NST = SK // P  # 16 s-tiles of x
SCALE = 1.0 / float(np.sqrt(E))

_cache: dict = {}

# Set by the first kernel() call; test harnesses can read .exec_time_ns etc.
LAST_RESULT = None


def _build():
    if "nc" in _cache:
        return _cache["nc"]

    import concourse.tile as tile
    from concourse import bacc, mybir
    from concourse.masks import make_identity

    ACT = mybir.ActivationFunctionType
    f32 = mybir.dt.float32
    f32r = mybir.dt.float32r

    def r(ap):
        return ap.bitcast(f32r)

    nc = bacc.Bacc("TRN2", target_bir_lowering=False, debug=False, num_devices=8)

    xq_d = nc.dram_tensor("xq", [SQ, D], f32, kind="ExternalInput").ap()
    xo_d = nc.dram_tensor("xo", [SQ, D], f32, kind="ExternalInput").ap()
    wq_d = nc.dram_tensor("wq", [D, E], f32, kind="ExternalInput").ap()
    wk_d = nc.dram_tensor("wk", [D, E], f32, kind="ExternalInput").ap()
    wv_d = nc.dram_tensor("wv", [D, E], f32, kind="ExternalInput").ap()
    bq_d = nc.dram_tensor("bq", [E], f32, kind="ExternalInput").ap()
    bk_d = nc.dram_tensor("bk", [E], f32, kind="ExternalInput").ap()
    bv_d = nc.dram_tensor("bv", [E], f32, kind="ExternalInput").ap()
    out_d = nc.dram_tensor("out", [SQ, E], f32, kind="ExternalOutput").ap()

    with tile.TileContext(nc) as tc:
        with (
            tc.tile_pool(name="const", bufs=1) as const,
            tc.tile_pool(name="xload", bufs=4) as xload,
            tc.tile_pool(name="big", bufs=1) as big,
            tc.tile_pool(name="exps", bufs=4) as exps,
            tc.tile_pool(name="misc", bufs=2) as misc,
            tc.tile_pool(name="ptr", bufs=2, space="PSUM") as ptr,
            tc.tile_pool(name="pacc", bufs=2, space="PSUM") as pacc,
            tc.tile_pool(name="po", bufs=2, space="PSUM") as po,
            tc.tile_pool(name="psum_s", bufs=1, space="PSUM") as psum_s,
        ):
            # ---- constants ----
            ident = const.tile([P, P], f32)
            make_identity(nc, ident)
            onesf = const.tile([P, 1], f32)
            nc.gpsimd.memset(onesf, 1.0)
            ones = const.tile([P, 1], f32r)
            nc.vector.tensor_copy(ones[:], onesf[:])
            w_sb = {}
            for name, wd in (("q", wq_d), ("k", wk_d), ("v", wv_d)):
                wf = const.tile([P, DC, E], f32, name=f"w{name}f")
                nc.sync.dma_start(wf[:], wd.rearrange("(t p) e -> p t e", p=P))
                w = const.tile([P, DC, E], f32r, name=f"w{name}")
                nc.vector.tensor_copy(w[:], wf[:])
                w_sb[name] = w
            b_sb = {}
            for name, bd in (("q", bq_d), ("k", bk_d), ("v", bv_d)):
                b = const.tile([P, 1], f32, name=f"b{name}")
                nc.sync.dma_start(b[:], bd[:, None])
                b_sb[name] = b

            # ---- big persistent tiles ----
            xT = big.tile([P, DC, SK], f32r)  # x^T: [d_lo, d_chunk, s] (q rows then o rows)
            qT = big.tile([P, SQ], f32r)  # Q^T: [e, q]
            kT = big.tile([P, SK], f32r)  # K^T: [e, k]
            vT = big.tile([P, SK], f32)  # V^T: [e, k] (staging)
            v_sb = big.tile([P, NST, E], f32r)  # V natural: [k_lo, k_chunk, e]
            out_sb = big.tile([P, SQ // P, E], f32)  # out: [q_lo, q_tile, e]

            # ---- phase 1: load x tiles, transpose into xT ----
            for st in range(NST):
                src = xq_d if st < NST // 2 else xo_d
                row0 = (st % (NST // 2)) * P
                xt_in = xload.tile([P, D], f32, tag="xin")
                nc.sync.dma_start(xt_in[:], src[row0 : row0 + P, :])
                for dc in range(DC):
                    ps = ptr.tile([P, P], f32, tag="tr")
                    nc.tensor.transpose(ps[:], xt_in[:, dc * P : (dc + 1) * P], ident[:])
                    dst = xT[:, dc, st * P : (st + 1) * P]
                    if (st + dc) % 2 == 0:
                        nc.vector.tensor_copy(dst, ps[:])
                    else:
                        nc.scalar.activation(dst, ps[:], ACT.Copy)

            # ---- phase 2: projections ----
            # Q^T over the first half of columns (this core's queries);
            # K^T / V^T over all columns.
            def proj(dst_col0, width, w, b, dst):
                psum = pacc.tile([P, QT], f32, tag="proj")
                for dc in range(DC):
                    nc.tensor.matmul(
                        psum[:, :width],
                        w[:, dc, :],
                        xT[:, dc, dst_col0 : dst_col0 + width],
                        start=(dc == 0),
                        stop=(dc == DC - 1),
                    )
                # copyback + per-partition bias (e lives on partitions)
                nc.scalar.activation(
                    dst[:, dst_col0 : dst_col0 + width],
                    psum[:, :width],
                    ACT.Identity,
                    bias=b[:],
                    scale=1.0,
                )

            for ct in range(SK // QT):
                col0 = ct * QT
                if ct < SQ // QT:
                    proj(col0, QT, w_sb["q"], b_sb["q"], qT)
                proj(col0, QT, w_sb["k"], b_sb["k"], kT)
                proj(col0, QT, w_sb["v"], b_sb["v"], vT)

            # ---- phase 3: V^T -> V natural layout ----
            for kc in range(NKC):
                ps = ptr.tile([P, P], f32, tag="tr")
                nc.tensor.transpose(ps[:], vT[:, kc * P : (kc + 1) * P], ident[:])
                nc.vector.tensor_copy(v_sb[:, kc, :], ps[:])

            # ---- phase 4: attention (transposed layout) ----
            for qt in range(NQT):
                q0 = qt * QT
                acc_o = po.tile([P, QT], f32, tag="acc_o")  # out^T accum [e, q]
                acc_s = psum_s.tile([1, QT], f32, tag="acc_s")  # softmax sums [1, q]
                for kc in range(NKC):
                    ps = pacc.tile([P, QT], f32, tag="proj")
                    nc.tensor.matmul(
                        ps[:],
                        kT[:, kc * P : (kc + 1) * P],
                        qT[:, q0 : q0 + QT],
                        start=True,
                        stop=True,
                    )
                    es = exps.tile([P, QT], f32r, tag="exps")
                    nc.scalar.activation(es[:], ps[:], ACT.Exp, scale=SCALE)
                    nc.tensor.matmul(
                        acc_o[:],
                        v_sb[:, kc, :],
                        es[:],
                        start=(kc == 0),
                        stop=(kc == NKC - 1),
                    )
                    nc.tensor.matmul(
                        acc_s[:],
                        ones[:],
                        es[:],
                        start=(kc == 0),
                        stop=(kc == NKC - 1),
                    )
                # normalize: out^T[e, q] / sums[q]
                sums_sb = misc.tile([1, QT], f32, tag="sums")
                nc.vector.tensor_copy(sums_sb[:], acc_s[:])
                recip = misc.tile([1, QT], f32, tag="recip")
                nc.vector.reciprocal(recip[:], sums_sb[:])
                rbc = misc.tile([P, QT], f32, tag="rbc")
                nc.gpsimd.partition_broadcast(rbc[:], recip[:])
                otn = misc.tile([P, QT], f32, tag="otn")
                nc.vector.tensor_mul(out=otn[:], in0=acc_o[:], in1=rbc[:])
                # transpose back to [q, e]
                for j in range(QT // P):
                    ps = ptr.tile([P, P], f32, tag="tr")
                    nc.tensor.transpose(ps[:], otn[:, j * P : (j + 1) * P], ident[:])
                    nc.vector.tensor_copy(out_sb[:, qt * (QT // P) + j, :], ps[:])

            nc.sync.dma_start(out_d.rearrange("(t p) e -> p t e", p=P), out_sb[:])

    nc.compile()
    _cache["nc"] = nc
    return nc


def kernel(x, Wq, bq, Wk, bk, Wv, bv):
    global LAST_RESULT
    nc = _build()
    from concourse import bass_utils

    x = np.asarray(x, dtype=np.float32)
    Wq = np.ascontiguousarray(np.asarray(Wq, dtype=np.float32))
    Wk = np.ascontiguousarray(np.asarray(Wk, dtype=np.float32))
    Wv = np.ascontiguousarray(np.asarray(Wv, dtype=np.float32))
    bq = np.ascontiguousarray(np.asarray(bq, dtype=np.float32))
    bk = np.ascontiguousarray(np.asarray(bk, dtype=np.float32))
    bv = np.ascontiguousarray(np.asarray(bv, dtype=np.float32))
    B, S, _ = x.shape

    in_maps = []
    for c in range(8):
        b, h = c // 2, c % 2
        xq = np.ascontiguousarray(x[b, h * SQ : (h + 1) * SQ])
        xo = np.ascontiguousarray(x[b, (1 - h) * SQ : (2 - h) * SQ])
        in_maps.append(
            {
                "xq": xq,
                "xo": xo,
                "wq": Wq,
                "wk": Wk,
                "wv": Wv,
                "bq": bq,
                "bk": bk,
                "bv": bv,
            }
        )

    res = bass_utils.run_bass_kernel_spmd(nc, in_maps, core_ids=list(range(8)))
    LAST_RESULT = res

    out = np.empty((B, S, E), dtype=np.float32)
    for c in range(8):
        b, h = c // 2, c % 2
        out[b, h * SQ : (h + 1) * SQ] = res.results[c]["out"]
    return out


# revision 11
# speedup vs baseline: 1.0397x; 1.0397x over previous
"""AttentionHead kernel for 8 Trainium2 NeuronCores.

Problem: x[4,2048,1024] -> Q/K/V projections (qkv_dim=128) -> softmax(Q K^T / sqrt(128)) @ V.

Sharding: core c handles batch b=c//2, query half h=c%2 (1024 queries), with the
full 2048-key sequence for that batch kept local (data-parallel over batch x
query-split; the SxS score matrix stays on-core per the hint). K/V rows are
processed in the order [this core's query half, other half] - softmax and the
attention-weighted sum are permutation-invariant over keys, so each core can
consume the two halves in its own order and no re-indexing is needed.

Pipeline per core (all layouts chosen so no operand ever needs a transposed
DMA): load x naturally [s,d]; PE-transpose to x^T [d,s]; projections
W.T @ x^T accumulate over 8 d-chunks into PSUM giving Q^T/K^T/V^T [e,s]
(bias fused into the ACT copyback); V^T re-transposed to natural V [k,e];
attention runs in the transposed layout: scores^T[k,q] = K^T-slice.T @ Q^T,
ACT exp with the 1/sqrt(128) scale fused (no max subtraction - scores are
~N(0,1) so exp cannot overflow), PV accumulates V.T @ expS^T over k-chunks in
PSUM alongside a ones-matmul accumulating the softmax denominators; normalize
once at the end and PE-transpose [e,q] -> [q,e] for the output.
"""

import os
import sys

if "/opt/trn_rl_repo" not in sys.path:
    sys.path.insert(0, "/opt/trn_rl_repo")

import numpy as np

P = 128
D = 1024  # d_model
DC = D // P  # 8 contraction chunks
E = 128  # qkv dim
SQ = 1024  # queries per core
SK = 2048  # keys per core
QT = 512  # query column-block width
NQT = SQ // QT  # 2
NKC = SK // P  # 16 key chunks
NST = SK // P  # 16 s-tiles of x
SCALE = 1.0 / float(np.sqrt(E))

# attention matmul dtype: "f32r" (more precise) or "bf16" (faster)
ATT = os.environ.get("KERNEL_ATT_DT", "bf16")

_cache: dict = {}

# Set by the first kernel() call; test harnesses can read .exec_time_ns etc.
LAST_RESULT = None


def _build():
    if "nc" in _cache:
        return _cache["nc"]

    import concourse.tile as tile
    from concourse import bacc, mybir
    from concourse.masks import make_identity

    ACTF = mybir.ActivationFunctionType
    f32 = mybir.dt.float32
    f32r = mybir.dt.float32r
    att_dt = mybir.dt.bfloat16 if ATT == "bf16" else f32r

    nc = bacc.Bacc("TRN2", target_bir_lowering=False, debug=False, num_devices=8)

    # x declared as float32r (same bytes as f32) so it can feed f32r
    # transpose-matmuls directly off the DMA.
    xq_d = nc.dram_tensor("xq", [SQ, D], f32r, kind="ExternalInput").ap()
    xo_d = nc.dram_tensor("xo", [SQ, D], f32r, kind="ExternalInput").ap()
    wq_d = nc.dram_tensor("wq", [D, E], f32, kind="ExternalInput").ap()
    wk_d = nc.dram_tensor("wk", [D, E], f32, kind="ExternalInput").ap()
    wv_d = nc.dram_tensor("wv", [D, E], f32, kind="ExternalInput").ap()
    bq_d = nc.dram_tensor("bq", [E], f32, kind="ExternalInput").ap()
    bk_d = nc.dram_tensor("bk", [E], f32, kind="ExternalInput").ap()
    bv_d = nc.dram_tensor("bv", [E], f32, kind="ExternalInput").ap()
    out_d = nc.dram_tensor("out", [SQ, E], f32, kind="ExternalOutput").ap()

    with tile.TileContext(nc) as tc:
        with (
            tc.tile_pool(name="const", bufs=1) as const,
            tc.tile_pool(name="xload", bufs=4) as xload,
            tc.tile_pool(name="big", bufs=1) as big,
            tc.tile_pool(name="exps", bufs=4) as exps,
            tc.tile_pool(name="misc", bufs=2) as misc,
            tc.tile_pool(name="ptr", bufs=2, space="PSUM") as ptr,
            tc.tile_pool(name="pacc", bufs=2, space="PSUM") as pacc,
            tc.tile_pool(name="po", bufs=2, space="PSUM") as po,
            tc.tile_pool(name="psum_s", bufs=1, space="PSUM") as psum_s,
        ):
            # ---- x loads first: they gate the whole PE pipeline ----
            x_in = []
            for st in range(NST):
                src = xq_d if st < NST // 2 else xo_d
                row0 = (st % (NST // 2)) * P
                xt_in = xload.tile([P, D], f32r, tag="xin")
                nc.sync.dma_start(xt_in[:], src[row0 : row0 + P, :])
                x_in.append(xt_in)

            # ---- constants (scalar-engine HWDGE queue; don't block x) ----
            identf = const.tile([P, P], f32)
            make_identity(nc, identf)
            ident = const.tile([P, P], f32r)
            nc.vector.tensor_copy(ident[:], identf[:])
            onesf = const.tile([P, 1], f32)
            nc.gpsimd.memset(onesf, 1.0)
            ones = const.tile([P, 1], att_dt)
            nc.vector.tensor_copy(ones[:], onesf[:])
            w_sb = {}
            for name, wd in (("q", wq_d), ("k", wk_d), ("v", wv_d)):
                wf = const.tile([P, DC, E], f32, name=f"w{name}f")
                nc.scalar.dma_start(wf[:], wd.rearrange("(t p) e -> p t e", p=P))
                w = const.tile([P, DC, E], f32r, name=f"w{name}")
                nc.vector.tensor_copy(w[:], wf[:])
                w_sb[name] = w
            b_sb = {}
            for name, bd in (("q", bq_d), ("k", bk_d), ("v", bv_d)):
                b = const.tile([P, 1], f32, name=f"b{name}")
                nc.scalar.dma_start(b[:], bd[:, None])
                b_sb[name] = b

            # ---- big persistent tiles ----
            xT = big.tile([P, DC, SK], f32r)  # x^T: [d_lo, d_chunk, s]
            qT = big.tile([P, SQ], att_dt)  # Q^T: [e, q]
            kT = big.tile([P, SK], att_dt)  # K^T: [e, k]
            vT = big.tile([P, SK], f32r)  # V^T: [e, k] (staging)
            v_sb = big.tile([P, NST, E], att_dt)  # V natural: [k_lo, k_chunk, e]

            # ---- phase 1: transpose x into xT (f32r transpose, 1.5 cyc/row) ----
            for st in range(NST):
                for dc in range(DC):
                    ps = ptr.tile([P, P], f32r, tag="tr")
                    nc.tensor.transpose(ps[:], x_in[st][:, dc * P : (dc + 1) * P], ident[:])
                    dst = xT[:, dc, st * P : (st + 1) * P]
                    if (st + dc) % 2 == 0:
                        nc.vector.tensor_copy(dst, ps[:])
                    else:
                        nc.scalar.activation(dst, ps[:], ACTF.Copy)

            # ---- phase 2: projections ----
            def proj(dst_col0, width, w, b, dst):
                psum = pacc.tile([P, QT], f32, tag="mm")
                for dc in range(DC):
                    nc.tensor.matmul(
                        psum[:, :width],
                        w[:, dc, :],
                        xT[:, dc, dst_col0 : dst_col0 + width],
                        start=(dc == 0),
                        stop=(dc == DC - 1),
                    )
                # copyback + per-partition bias (e lives on partitions)
                nc.scalar.activation(
                    dst[:, dst_col0 : dst_col0 + width],
                    psum[:, :width],
                    ACTF.Identity,
                    bias=b[:],
                    scale=1.0,
                )

            for ct in range(SK // QT):
                col0 = ct * QT
                if ct < SQ // QT:
                    proj(col0, QT, w_sb["q"], b_sb["q"], qT)
                proj(col0, QT, w_sb["k"], b_sb["k"], kT)
                proj(col0, QT, w_sb["v"], b_sb["v"], vT)

            # ---- phase 3: V^T -> V natural layout ----
            for kc in range(NKC):
                ps = ptr.tile([P, P], f32r, tag="tr")
                nc.tensor.transpose(ps[:], vT[:, kc * P : (kc + 1) * P], ident[:])
                nc.vector.tensor_copy(v_sb[:, kc, :], ps[:])

            # ---- phase 4: attention (transposed layout) ----
            for qt in range(NQT):
                q0 = qt * QT
                acc_o = po.tile([P, QT], f32, tag="acc_o")  # out^T accum [e, q]
                acc_s = psum_s.tile([1, QT], f32, tag="acc_s")  # softmax sums
                for kc in range(NKC):
                    ps = pacc.tile([P, QT], f32, tag="mm")
                    nc.tensor.matmul(
                        ps[:],
                        kT[:, kc * P : (kc + 1) * P],
                        qT[:, q0 : q0 + QT],
                        start=True,
                        stop=True,
                    )
                    es = exps.tile([P, QT], att_dt, tag="exps")
                    nc.scalar.activation(es[:], ps[:], ACTF.Exp, scale=SCALE)
                    nc.tensor.matmul(
                        acc_o[:],
                        v_sb[:, kc, :],
                        es[:],
                        start=(kc == 0),
                        stop=(kc == NKC - 1),
                    )
                    nc.tensor.matmul(
                        acc_s[:],
                        ones[:],
                        es[:],
                        start=(kc == 0),
                        stop=(kc == NKC - 1),
                    )
                # normalize: out^T[e, q] / sums[q]
                sums_sb = misc.tile([1, QT], f32, tag="sums")
                nc.vector.tensor_copy(sums_sb[:], acc_s[:])
                recip = misc.tile([1, QT], f32, tag="recip")
                nc.vector.reciprocal_approx_fast(recip[:], sums_sb[:])
                rbc = misc.tile([P, QT], f32, tag="rbc")
                nc.gpsimd.partition_broadcast(rbc[:], recip[:])
                otn = misc.tile([P, QT], f32, tag="otn")
                nc.vector.tensor_mul(out=otn[:], in0=acc_o[:], in1=rbc[:])
                # transpose back to [q, e] and store
                out_sb = misc.tile([P, QT // P, E], f32, tag="outsb")
                for j in range(QT // P):
                    ps = ptr.tile([P, P], f32, tag="tr")
                    nc.tensor.transpose(ps[:], otn[:, j * P : (j + 1) * P], identf[:])
                    nc.vector.tensor_copy(out_sb[:, j, :], ps[:])
                nc.sync.dma_start(
                    out_d[q0 : q0 + QT, :].rearrange("(t p) e -> p t e", p=P),
                    out_sb[:],
                )

    nc.compile()
    _cache["nc"] = nc
    return nc


def kernel(x, Wq, bq, Wk, bk, Wv, bv):
    global LAST_RESULT
    nc = _build()
    from concourse import bass_utils

    x = np.asarray(x, dtype=np.float32)
    Wq = np.ascontiguousarray(np.asarray(Wq, dtype=np.float32))
    Wk = np.ascontiguousarray(np.asarray(Wk, dtype=np.float32))
    Wv = np.ascontiguousarray(np.asarray(Wv, dtype=np.float32))
    bq = np.ascontiguousarray(np.asarray(bq, dtype=np.float32))
    bk = np.ascontiguousarray(np.asarray(bk, dtype=np.float32))
    bv = np.ascontiguousarray(np.asarray(bv, dtype=np.float32))
    B, S, _ = x.shape

    in_maps = []
    for c in range(8):
        b, h = c // 2, c % 2
        xq = np.ascontiguousarray(x[b, h * SQ : (h + 1) * SQ])
        xo = np.ascontiguousarray(x[b, (1 - h) * SQ : (2 - h) * SQ])
        in_maps.append(
            {
                "xq": xq,
                "xo": xo,
                "wq": Wq,
                "wk": Wk,
                "wv": Wv,
                "bq": bq,
                "bk": bk,
                "bv": bv,
            }
        )

    res = bass_utils.run_bass_kernel_spmd(nc, in_maps, core_ids=list(range(8)))
    LAST_RESULT = res

    out = np.empty((B, S, E), dtype=np.float32)
    for c in range(8):
        b, h = c // 2, c % 2
        out[b, h * SQ : (h + 1) * SQ] = res.results[c]["out"]
    return out
